# revision 1
# baseline (speedup 1.0000x reference)
"""Trainium2 Bass kernel for nn_JointConditionalDistributionBlock.

Math (see analysis):
  output = softmax(marginals(m_k), axis=1), where
  m_k[h1,h2,h3] = sum_{f1..f4} softmax_{f4}(j_k + B)[h,f] * P_X[f]
The KDE scalar j_k is constant over the whole tensor, and softmax is
shift-invariant, so it drops out exactly:  softmax(j_k + B) == softmax(B).
P_X = softmax_{f4}(outer(x + tpx_bias) + bias_X) is a tiny [12^4] table.

Device work = stream B = bias_Y_given_X ([12]^7 f32, ~143 MB) and compute,
per 12-wide row r=(h,f1,f2,f3):
    num(r) = sum_f4 exp(B[r,f4]) * px[f123,f4]
    den(r) = sum_f4 exp(B[r,f4])
    m(h)   = sum_{f123} num/den
Sharding: 1728 h-triples / 8 cores = 216 triples (17.9 MB) per core.

Layout trick: the host pre-transposes each shard so the softmax axis f4
sits on SBUF partitions (partition = t_local*12 + f4, free = (f1,f2,f3)).
The grouped sums over f4 then run on the TensorEngine as matmuls with a
block-diagonal ones stationary: W_s[(t,f4), 10s+t] = 1.  Twelve tiles
accumulate into one [120,1728] PSUM pair (den banks 0-3, num banks 4-7)
via start=False, so the softmax-normalize stage runs on full 120-partition
tiles: reciprocal_approx_fast + mul + row-sum reduce.

DMA: per-dma_start fixed cost (~2us completion receipt) serializes on one
HWDGE FIFO, so tiles are loaded in PAIRS (1.66 MB per dma_start) and the
loads alternate between nc.sync (HWDGE) and nc.gpsimd (SWDGE) so the two
descriptor paths overlap each other's fixed latency.
"""

import numpy as np

H_P, F_P, K = 3, 4, 12
D = H_P + F_P
N_CORES = 8
NTRIP = K ** H_P            # 1728 h-triples total
TPC = NTRIP // N_CORES      # 216 triples per core
FREE = K ** 3               # 1728 = (f1,f2,f3)
TPT = 10                    # triples per full tile -> 120 partitions
ROWS_FULL = TPT * K         # 120
# superblocks: slot lists of triples-per-tile
SBS = [[TPT] * 12, [TPT] * 9 + [6]]     # 120 + 96 = 216
CHUNKS = [(0, 512), (512, 512), (1024, 512), (1536, 192)]

_CACHE = {}


def _softmax_last(x):
    x = np.asarray(x, np.float32)
    m = x.max(axis=-1, keepdims=True)
    e = np.exp(x - m, dtype=np.float32)
    return e / e.sum(axis=-1, keepdims=True)


def _build_program(reps=1, loop_n=1, variant="full"):
    import contextlib

    import concourse.bacc as bacc
    from concourse import mybir
    from concourse.tile import TileContext

    nc = bacc.Bacc("TRN2", target_bir_lowering=False, debug=False)
    f32 = mybir.dt.float32
    bf16 = mybir.dt.bfloat16

    xin = nc.dram_tensor("xin", [TPC * K, FREE], f32, kind="ExternalInput").ap()
    pxr = nc.dram_tensor("pxr", [ROWS_FULL, FREE], bf16, kind="ExternalInput").ap()
    wst = nc.dram_tensor("wst", [ROWS_FULL, 13, ROWS_FULL], bf16,
                         kind="ExternalInput").ap()
    mout = nc.dram_tensor("mout", [TPC, 1], f32, kind="ExternalOutput").ap()

    with TileContext(nc) as tc:
        with (
            tc.tile_pool(name="singles", bufs=1) as singles,
            tc.tile_pool(name="xp", bufs=4) as xp,
            tc.tile_pool(name="ep", bufs=3) as epool,
            tc.tile_pool(name="epp", bufs=3) as eppool,
            tc.tile_pool(name="qp", bufs=2) as qp,
            tc.tile_pool(name="ps", bufs=1, space="PSUM") as ps,
        ):
            px_s = singles.tile([ROWS_FULL, FREE], bf16)
            nc.sync.dma_start(out=px_s, in_=pxr)
            w_s = singles.tile([ROWS_FULL, 13, ROWS_FULL], bf16)
            nc.sync.dma_start(out=w_s, in_=wst)

            loop_cm = (tc.For_i(0, loop_n, 1) if loop_n > 1
                       else contextlib.nullcontext())
            with loop_cm:
              for _rep in range(reps):
                row = 0
                trip = 0
                dma_i = 0
                cons_i = 0
                for slots in SBS:
                    ntrip_sb = sum(slots)
                    den_p = ps.tile([ROWS_FULL, FREE], mybir.dt.float32)
                    num_p = ps.tile([ROWS_FULL, FREE], mybir.dt.float32)
                    last = len(slots) - 1
                    # group slots into pairs; each full pair = one DMA
                    for p0 in range(0, len(slots), 2):
                        pair = slots[p0:p0 + 2]
                        x2_t = xp.tile([ROWS_FULL, 2, FREE], f32)
                        eng = (nc.sync, nc.gpsimd)[dma_i % 2]
                        dma_i += 1
                        full_pair = (len(pair) == 2 and pair[0] == TPT
                                     and pair[1] == TPT)
                        if full_pair:
                            src = xin[row:row + 2 * ROWS_FULL, :].rearrange(
                                "(two p) f -> p two f", two=2)
                            eng.dma_start(out=x2_t, in_=src)
                        else:
                            for k_, tpt in enumerate(pair):
                                P = tpt * K
                                r0 = row + k_ * pair[0] * K
                                eng2 = (nc.sync, nc.gpsimd)[dma_i % 2]
                                dma_i += 1
                                eng2.dma_start(out=x2_t[:P, k_, :],
                                               in_=xin[r0:r0 + P, :])
                        if variant == "dmaonly":
                            c_t = qp.tile([ROWS_FULL, 1], mybir.dt.float32,
                                          tag="consume")
                            nc.vector.tensor_reduce(
                                out=c_t[:2], in_=x2_t[:2, 0, :8],
                                axis=mybir.AxisListType.X,
                                op=mybir.AluOpType.add)
                            nc.sync.dma_start(
                                out=mout[2 * cons_i:2 * cons_i + 2, :],
                                in_=c_t[:2])
                            cons_i += 1
                            row += sum(t * K for t in pair)
                            continue
                        e2_t = epool.tile([ROWS_FULL, 2, FREE], bf16)
                        ep2_t = eppool.tile([ROWS_FULL, 2, FREE], bf16)
                        if full_pair:
                            nc.scalar.activation(
                                out=e2_t, in_=x2_t,
                                func=mybir.ActivationFunctionType.Exp)
                            if variant != "nodvemul":
                                nc.vector.tensor_mul(
                                    ep2_t[:, 0, :], e2_t[:, 0, :], px_s)
                                nc.vector.tensor_mul(
                                    ep2_t[:, 1, :], e2_t[:, 1, :], px_s)
                        else:
                            for k_, tpt in enumerate(pair):
                                P = tpt * K
                                nc.scalar.activation(
                                    out=e2_t[:P, k_, :], in_=x2_t[:P, k_, :],
                                    func=mybir.ActivationFunctionType.Exp)
                                if variant != "nodvemul":
                                    nc.vector.tensor_mul(
                                        ep2_t[:P, k_, :], e2_t[:P, k_, :],
                                        px_s[:P])
                        if variant == "nodvemul":
                            ep2_t = e2_t
                        if variant == "nope":
                            c_t = qp.tile([ROWS_FULL, 1], mybir.dt.float32,
                                          tag="consume")
                            nc.vector.tensor_reduce(
                                out=c_t[:2], in_=ep2_t[:2, 0, :8],
                                axis=mybir.AxisListType.X,
                                op=mybir.AluOpType.add)
                            nc.sync.dma_start(
                                out=mout[2 * cons_i:2 * cons_i + 2, :],
                                in_=c_t[:2])
                            cons_i += 1
                            row += sum(t * K for t in pair)
                            continue
                        for k_, tpt in enumerate(pair):
                            s = p0 + k_
                            P = tpt * K
                            w_idx = 12 if tpt == 6 else s
                            lhsT = w_s[:P, w_idx, :]
                            for c0, cn in CHUNKS:
                                nc.tensor.matmul(
                                    den_p[:, c0:c0 + cn], lhsT,
                                    e2_t[:P, k_, c0:c0 + cn],
                                    start=(s == 0), stop=(s == last))
                                nc.tensor.matmul(
                                    num_p[:, c0:c0 + cn], lhsT,
                                    ep2_t[:P, k_, c0:c0 + cn],
                                    start=(s == 0), stop=(s == last))
                        row += sum(t * K for t in pair)
                    if variant in ("dmaonly", "nope"):
                        trip += ntrip_sb
                        continue
                    recip_t = qp.tile([ROWS_FULL, FREE], mybir.dt.float32)
                    nc.vector.reciprocal_approx_fast(
                        out=recip_t[:ntrip_sb], in_=den_p[:ntrip_sb])
                    qv_t = qp.tile([ROWS_FULL, FREE], mybir.dt.float32)
                    m_t = qp.tile([ROWS_FULL, 1], mybir.dt.float32)
                    nc.vector.tensor_mul(qv_t[:ntrip_sb], num_p[:ntrip_sb],
                                         recip_t[:ntrip_sb])
                    nc.vector.tensor_reduce(
                        out=m_t[:ntrip_sb], in_=qv_t[:ntrip_sb],
                        axis=mybir.AxisListType.X, op=mybir.AluOpType.add)
                    nc.sync.dma_start(out=mout[trip:trip + ntrip_sb, :],
                                      in_=m_t[:ntrip_sb])
                    trip += ntrip_sb

    nc.compile()
    return nc


def _host_tables(x, tpx_bias, bias_X):
    import ml_dtypes

    t = (np.asarray(x, np.float32) + np.asarray(tpx_bias, np.float32)[0])
    r = t[0]
    for n in range(1, F_P):
        r = r[..., None] * t[n]                      # [12,12,12,12]
    px = _softmax_last(r + np.asarray(bias_X, np.float32))
    pxT = np.ascontiguousarray(px.transpose(3, 0, 1, 2)).reshape(K, FREE)
    pxr = np.ascontiguousarray(np.tile(pxT, (TPT, 1))).astype(ml_dtypes.bfloat16)

    W = np.zeros((13, ROWS_FULL, ROWS_FULL), np.float32)
    for s in range(12):
        for t_ in range(TPT):
            W[s, t_ * K:(t_ + 1) * K, 10 * s + t_] = 1.0
    for t_ in range(6):
        W[12, t_ * K:(t_ + 1) * K, 90 + t_] = 1.0
    wst = np.ascontiguousarray(W.transpose(1, 0, 2)).astype(ml_dtypes.bfloat16)
    return pxr, wst


def kernel(x, context_x, context_y, H_bandwidth, tpx_bias, bias_Y_given_X,
           bias_X):
    from concourse.bass_utils import run_bass_kernel_spmd

    if "nc" not in _CACHE:
        _CACHE["nc"] = _build_program()
    nc = _CACHE["nc"]

    pxr, wst = _host_tables(x, tpx_bias, bias_X)

    B7 = np.ascontiguousarray(np.asarray(bias_Y_given_X, np.float32)).reshape(
        NTRIP, K, K, K, K)
    in_maps = []
    for c in range(N_CORES):
        shard = B7[c * TPC:(c + 1) * TPC]            # [216, f1,f2,f3,f4]
        xc = np.ascontiguousarray(shard.transpose(0, 4, 1, 2, 3)).reshape(
            TPC * K, FREE)                           # row = t*12+f4
        in_maps.append({"xin": xc, "pxr": pxr, "wst": wst})

    res = run_bass_kernel_spmd(nc, in_maps, list(range(N_CORES)))
    m_flat = np.concatenate(
        [np.asarray(res.results[c]["mout"], np.float32)[:, 0]
         for c in range(N_CORES)])
    m_k = m_flat.reshape(K, K, K)

    marginals = np.stack([
        m_k.sum(axis=(1, 2)), m_k.sum(axis=(0, 2)), m_k.sum(axis=(0, 1))
    ]).astype(np.float32)
    return _softmax_last(marginals).astype(np.float32)



# revision 3
# speedup vs baseline: 17955.3632x; 17955.3632x over previous
"""Trainium2 Bass kernel for nn_JointConditionalDistributionBlock.

Math (see analysis):
  output = softmax(marginals(m_k), axis=1), where
  m_k[h1,h2,h3] = sum_{f1..f4} softmax_{f4}(j_k + B)[h,f] * P_X[f]
The KDE scalar j_k is constant over the whole tensor, and softmax is
shift-invariant, so it drops out exactly:  softmax(j_k + B) == softmax(B).
P_X = softmax_{f4}(outer(x + tpx_bias) + bias_X) is a tiny [12^4] table.

Device work = stream B = bias_Y_given_X ([12]^7, ~143 MB) and compute,
per 12-wide row r=(h,f1,f2,f3):
    num(r) = sum_f4 exp(B[r,f4]) * px[f123,f4]
    den(r) = sum_f4 exp(B[r,f4])
    m(h)   = sum_{f123} num/den
Sharding: 1728 h-triples / 8 cores = 216 triples per core.

v2 layout: host converts the stream to bf16 (halves HBM traffic; the
normalizing division + averaging over 20k terms keeps rel err far under
tolerance) and pre-arranges each core's shard as one [120, 22*1728] bf16
matrix whose partition dim is t_local*12 + f4 (softmax axis on SBUF
partitions) and whose free dim concatenates the 22 tiles' (f1,f2,f3)
planes. Each dma_start then moves a 4-tile group as one fully-contiguous
13.8 KB-per-partition segment (~1.66 MB), amortizing the ~2us DMA
completion receipt; loads alternate HWDGE (nc.sync) / SWDGE (nc.gpsimd)
so the two descriptor paths overlap. Partial tiles are zero-padded on
host (exp(0)=1 rows are never touched by the matmul's [:P] views and the
ScalarE cost is free-dim-bound, so padding is free).

Grouped f4-sums run on the TensorEngine: block-diagonal ones stationary
per slot, accumulated over 12 slots into a [120,1728] PSUM pair
(den banks 0-3, num banks 4-7); normalize = reciprocal_approx_fast +
mul + row-sum on VectorE.
"""

import numpy as np

H_P, F_P, K = 3, 4, 12
D = H_P + F_P
N_CORES = 8
NTRIP = K ** H_P            # 1728 h-triples total
TPC = NTRIP // N_CORES      # 216 triples per core
FREE = K ** 3               # 1728 = (f1,f2,f3)
TPT = 10                    # triples per full tile -> 120 partitions
ROWS_FULL = TPT * K         # 120
# superblocks: slot lists of triples-per-tile
SBS = [[TPT] * 12, [TPT] * 9 + [6]]     # 120 + 96 = 216
CHUNKS = [(0, 512), (512, 512), (1024, 512), (1536, 192)]
# DMA groups: lists of (superblock, first slot, n slots)
DMA_GROUPS = [(0, 0, 4), (0, 4, 4), (0, 8, 4),
              (1, 0, 4), (1, 4, 4), (1, 8, 2)]
NTILES = 22

_CACHE = {}


def _softmax_last(x):
    x = np.asarray(x, np.float32)
    m = x.max(axis=-1, keepdims=True)
    e = np.exp(x - m, dtype=np.float32)
    return e / e.sum(axis=-1, keepdims=True)


def _build_program():
    import concourse.bacc as bacc
    from concourse import mybir
    from concourse.tile import TileContext

    nc = bacc.Bacc("TRN2", target_bir_lowering=False, debug=False)
    f32 = mybir.dt.float32
    bf16 = mybir.dt.bfloat16

    # xin: [partition(=t_local*12+f4), tile-concatenated f123 planes]
    xin = nc.dram_tensor("xin", [ROWS_FULL, NTILES * FREE], bf16,
                         kind="ExternalInput").ap()
    pxr = nc.dram_tensor("pxr", [ROWS_FULL, FREE], bf16, kind="ExternalInput").ap()
    wst = nc.dram_tensor("wst", [ROWS_FULL, 13, ROWS_FULL], bf16,
                         kind="ExternalInput").ap()
    mout = nc.dram_tensor("mout", [TPC, 1], f32, kind="ExternalOutput").ap()

    def finish_sb(den_p, num_p, ntrip_sb, trip):
        recip_t = qp.tile([ROWS_FULL, FREE], f32)
        nc.vector.reciprocal_approx_fast(
            out=recip_t[:ntrip_sb], in_=den_p[:ntrip_sb])
        qv_t = qp.tile([ROWS_FULL, FREE], f32)
        m_t = qp.tile([ROWS_FULL, 1], f32)
        nc.vector.tensor_mul(qv_t[:ntrip_sb], num_p[:ntrip_sb],
                             recip_t[:ntrip_sb])
        nc.vector.tensor_reduce(
            out=m_t[:ntrip_sb], in_=qv_t[:ntrip_sb],
            axis=mybir.AxisListType.X, op=mybir.AluOpType.add)
        nc.sync.dma_start(out=mout[trip:trip + ntrip_sb, :],
                          in_=m_t[:ntrip_sb])
        return trip + ntrip_sb

    with TileContext(nc) as tc:
        with (
            tc.tile_pool(name="singles", bufs=1) as singles,
            tc.tile_pool(name="xp", bufs=3) as xp,
            tc.tile_pool(name="ep", bufs=3) as epool,
            tc.tile_pool(name="epp", bufs=3) as eppool,
            tc.tile_pool(name="qp", bufs=2) as qp,
            tc.tile_pool(name="ps", bufs=1, space="PSUM") as ps,
        ):
            px_s = singles.tile([ROWS_FULL, FREE], bf16)
            nc.sync.dma_start(out=px_s, in_=pxr)
            w_s = singles.tile([ROWS_FULL, 13, ROWS_FULL], bf16)
            nc.sync.dma_start(out=w_s, in_=wst)

            tile0 = 0
            trip = 0
            den_p = num_p = None
            cur_sb = -1
            for sb, s0, ns in DMA_GROUPS:
                slots = SBS[sb]
                if sb != cur_sb:
                    if cur_sb >= 0:
                        trip = finish_sb(den_p, num_p, sum(SBS[cur_sb]), trip)
                    den_p = ps.tile([ROWS_FULL, FREE], f32)
                    num_p = ps.tile([ROWS_FULL, FREE], f32)
                    cur_sb = sb
                x_t = xp.tile([ROWS_FULL, ns, FREE], bf16)
                eng = (nc.sync, nc.gpsimd)[tile0 % 2]
                eng.dma_start(
                    out=x_t,
                    in_=xin[:, tile0 * FREE:(tile0 + ns) * FREE].rearrange(
                        "p (n f) -> p n f", n=ns))
                tile0 += ns
                e_t = epool.tile([ROWS_FULL, ns, FREE], bf16)
                ep_t = eppool.tile([ROWS_FULL, ns, FREE], bf16)
                nc.scalar.activation(
                    out=e_t, in_=x_t, func=mybir.ActivationFunctionType.Exp)
                for j in range(ns):
                    nc.vector.tensor_mul(ep_t[:, j, :], e_t[:, j, :], px_s)
                last = len(slots) - 1
                for j in range(ns):
                    s = s0 + j
                    tpt = slots[s]
                    P = tpt * K
                    w_idx = 12 if tpt == 6 else s
                    lhsT = w_s[:P, w_idx, :]
                    for c0, cn in CHUNKS:
                        nc.tensor.matmul(
                            den_p[:, c0:c0 + cn], lhsT,
                            e_t[:P, j, c0:c0 + cn],
                            start=(s == 0), stop=(s == last))
                        nc.tensor.matmul(
                            num_p[:, c0:c0 + cn], lhsT,
                            ep_t[:P, j, c0:c0 + cn],
                            start=(s == 0), stop=(s == last))
            trip = finish_sb(den_p, num_p, sum(SBS[cur_sb]), trip)

    nc.compile()
    return nc


def _host_tables(x, tpx_bias, bias_X):
    import ml_dtypes

    t = (np.asarray(x, np.float32) + np.asarray(tpx_bias, np.float32)[0])
    r = t[0]
    for n in range(1, F_P):
        r = r[..., None] * t[n]                      # [12,12,12,12]
    px = _softmax_last(r + np.asarray(bias_X, np.float32))
    pxT = np.ascontiguousarray(px.transpose(3, 0, 1, 2)).reshape(K, FREE)
    pxr = np.ascontiguousarray(np.tile(pxT, (TPT, 1))).astype(ml_dtypes.bfloat16)

    W = np.zeros((13, ROWS_FULL, ROWS_FULL), np.float32)
    for s in range(12):
        for t_ in range(TPT):
            W[s, t_ * K:(t_ + 1) * K, 10 * s + t_] = 1.0
    for t_ in range(6):
        W[12, t_ * K:(t_ + 1) * K, 90 + t_] = 1.0
    wst = np.ascontiguousarray(W.transpose(1, 0, 2)).astype(ml_dtypes.bfloat16)
    return pxr, wst


def _shard_inputs(bias_Y_given_X):
    """Per-core [120, 22*1728] bf16: partition = t_local*12 + f4, free =
    concatenated tile f123 planes (tile 21 zero-padded past 6 triples)."""
    import ml_dtypes

    B7 = np.ascontiguousarray(np.asarray(bias_Y_given_X, np.float32)).reshape(
        NTRIP, K, K, K, K)
    tpts = [tpt for slots in SBS for tpt in slots]
    xs = []
    for c in range(N_CORES):
        shard = B7[c * TPC:(c + 1) * TPC]            # [216, f1,f2,f3,f4]
        rows = np.ascontiguousarray(shard.transpose(0, 4, 1, 2, 3)).reshape(
            TPC * K, FREE)                           # row = t*12+f4
        flat = np.zeros((ROWS_FULL, NTILES, FREE), np.float32)
        r = 0
        for ti, tpt in enumerate(tpts):
            P = tpt * K
            flat[:P, ti] = rows[r:r + P]
            r += P
        xs.append(flat.reshape(ROWS_FULL, NTILES * FREE)
                  .astype(ml_dtypes.bfloat16))
    return xs


def kernel(x, context_x, context_y, H_bandwidth, tpx_bias, bias_Y_given_X,
           bias_X):
    from concourse.bass_utils import run_bass_kernel_spmd

    if "nc" not in _CACHE:
        _CACHE["nc"] = _build_program()
    nc = _CACHE["nc"]

    pxr, wst = _host_tables(x, tpx_bias, bias_X)
    xs = _shard_inputs(bias_Y_given_X)
    in_maps = [{"xin": xs[c], "pxr": pxr, "wst": wst} for c in range(N_CORES)]

    res = run_bass_kernel_spmd(nc, in_maps, list(range(N_CORES)))
    m_flat = np.concatenate(
        [np.asarray(res.results[c]["mout"], np.float32)[:, 0]
         for c in range(N_CORES)])
    m_k = m_flat.reshape(K, K, K)

    marginals = np.stack([
        m_k.sum(axis=(1, 2)), m_k.sum(axis=(0, 2)), m_k.sum(axis=(0, 1))
    ]).astype(np.float32)
    return _softmax_last(marginals).astype(np.float32)


# revision 4
# speedup vs baseline: 25269.2115x; 1.4073x over previous
"""Trainium2 Bass kernel for nn_JointConditionalDistributionBlock.

Math (see analysis):
  output = softmax(marginals(m_k), axis=1), where
  m_k[h1,h2,h3] = sum_{f1..f4} softmax_{f4}(j_k + B)[h,f] * P_X[f]
The KDE scalar j_k is constant over the whole tensor, and softmax is
shift-invariant, so it drops out exactly:  softmax(j_k + B) == softmax(B).
P_X = softmax_{f4}(outer(x + tpx_bias) + bias_X) is a tiny [12^4] table.

Device work = stream B = bias_Y_given_X ([12]^7, ~143 MB) and compute,
per 12-wide row r=(h,f1,f2,f3):
    num(r) = sum_f4 exp(B[r,f4]) * px[f123,f4]
    den(r) = sum_f4 exp(B[r,f4])
    m(h)   = sum_{f123} num/den
Sharding: 1728 h-triples / 8 cores = 216 triples per core.

v2 layout: host converts the stream to bf16 (halves HBM traffic; the
normalizing division + averaging over 20k terms keeps rel err far under
tolerance) and pre-arranges each core's shard as one [120, 22*1728] bf16
matrix whose partition dim is t_local*12 + f4 (softmax axis on SBUF
partitions) and whose free dim concatenates the 22 tiles' (f1,f2,f3)
planes. Each dma_start then moves a 4-tile group as one fully-contiguous
13.8 KB-per-partition segment (~1.66 MB), amortizing the ~2us DMA
completion receipt; loads alternate HWDGE (nc.sync) / SWDGE (nc.gpsimd)
so the two descriptor paths overlap. Partial tiles are zero-padded on
host (exp(0)=1 rows are never touched by the matmul's [:P] views and the
ScalarE cost is free-dim-bound, so padding is free).

Grouped f4-sums run on the TensorEngine: block-diagonal ones stationary
per slot, accumulated over 12 slots into a [120,1728] PSUM pair
(den banks 0-3, num banks 4-7); normalize = reciprocal_approx_fast +
mul + row-sum on VectorE.
"""

import numpy as np

H_P, F_P, K = 3, 4, 12
D = H_P + F_P
N_CORES = 8
NTRIP = K ** H_P            # 1728 h-triples total
TPC = NTRIP // N_CORES      # 216 triples per core
FREE = K ** 3               # 1728 = (f1,f2,f3)
TPT = 10                    # triples per full tile -> 120 partitions
ROWS_FULL = TPT * K         # 120
# superblocks: slot lists of triples-per-tile
SBS = [[TPT] * 12, [TPT] * 9 + [6]]     # 120 + 96 = 216
CHUNKS = [(0, 512), (512, 512), (1024, 512), (1536, 192)]
# DMA groups: lists of (superblock, first slot, n slots)
DMA_GROUPS = [(0, 0, 4), (0, 4, 4), (0, 8, 4),
              (1, 0, 4), (1, 4, 4), (1, 8, 2)]
NTILES = 22

_CACHE = {}


def _softmax_last(x):
    x = np.asarray(x, np.float32)
    m = x.max(axis=-1, keepdims=True)
    e = np.exp(x - m, dtype=np.float32)
    return e / e.sum(axis=-1, keepdims=True)


def _build_program():
    import concourse.bacc as bacc
    from concourse import mybir
    from concourse.tile import TileContext

    nc = bacc.Bacc("TRN2", target_bir_lowering=False, debug=False)
    f32 = mybir.dt.float32
    bf16 = mybir.dt.bfloat16

    # xin: [partition(=t_local*12+f4), tile-concatenated f123 planes]
    xin = nc.dram_tensor("xin", [ROWS_FULL, NTILES * FREE], bf16,
                         kind="ExternalInput").ap()
    pxr = nc.dram_tensor("pxr", [ROWS_FULL, FREE], bf16, kind="ExternalInput").ap()
    wst = nc.dram_tensor("wst", [ROWS_FULL, 13, ROWS_FULL], bf16,
                         kind="ExternalInput").ap()
    mout = nc.dram_tensor("mout", [TPC, 1], f32, kind="ExternalOutput").ap()

    def finish_sb(den_p, num_p, ntrip_sb, trip):
        recip_t = qp.tile([ROWS_FULL, FREE], f32)
        nc.vector.reciprocal_approx_fast(
            out=recip_t[:ntrip_sb], in_=den_p[:ntrip_sb])
        qv_t = qp.tile([ROWS_FULL, FREE], f32)
        m_t = qp.tile([ROWS_FULL, 1], f32)
        nc.vector.tensor_mul(qv_t[:ntrip_sb], num_p[:ntrip_sb],
                             recip_t[:ntrip_sb])
        nc.vector.tensor_reduce(
            out=m_t[:ntrip_sb], in_=qv_t[:ntrip_sb],
            axis=mybir.AxisListType.X, op=mybir.AluOpType.add)
        nc.sync.dma_start(out=mout[trip:trip + ntrip_sb, :],
                          in_=m_t[:ntrip_sb])
        return trip + ntrip_sb

    with TileContext(nc) as tc:
        with (
            tc.tile_pool(name="singles", bufs=1) as singles,
            tc.tile_pool(name="xp", bufs=3) as xp,
            tc.tile_pool(name="ep", bufs=3) as epool,
            tc.tile_pool(name="epp", bufs=3) as eppool,
            tc.tile_pool(name="qp", bufs=2) as qp,
            tc.tile_pool(name="ps", bufs=1, space="PSUM") as ps,
        ):
            px_s = singles.tile([ROWS_FULL, FREE], bf16)
            nc.sync.dma_start(out=px_s, in_=pxr)
            w_s = singles.tile([ROWS_FULL, 13, ROWS_FULL], bf16)
            nc.sync.dma_start(out=w_s, in_=wst)

            tile0 = 0
            trip = 0
            den_p = num_p = None
            cur_sb = -1
            for sb, s0, ns in DMA_GROUPS:
                slots = SBS[sb]
                if sb != cur_sb:
                    if cur_sb >= 0:
                        trip = finish_sb(den_p, num_p, sum(SBS[cur_sb]), trip)
                    den_p = ps.tile([ROWS_FULL, FREE], f32)
                    num_p = ps.tile([ROWS_FULL, FREE], f32)
                    cur_sb = sb
                x_t = xp.tile([ROWS_FULL, ns, FREE], bf16)
                eng = (nc.sync, nc.gpsimd)[tile0 % 2]
                eng.dma_start(
                    out=x_t,
                    in_=xin[:, tile0 * FREE:(tile0 + ns) * FREE].rearrange(
                        "p (n f) -> p n f", n=ns))
                tile0 += ns
                e_t = epool.tile([ROWS_FULL, ns, FREE], bf16)
                ep_t = eppool.tile([ROWS_FULL, ns, FREE], bf16)
                nc.scalar.activation(
                    out=e_t, in_=x_t, func=mybir.ActivationFunctionType.Exp)
                for j in range(ns):
                    nc.vector.tensor_mul(ep_t[:, j, :], e_t[:, j, :], px_s)
                last = len(slots) - 1
                for j in range(ns):
                    s = s0 + j
                    tpt = slots[s]
                    P = tpt * K
                    w_idx = 12 if tpt == 6 else s
                    lhsT = w_s[:P, w_idx, :]
                    for c0, cn in CHUNKS:
                        nc.tensor.matmul(
                            den_p[:, c0:c0 + cn], lhsT,
                            e_t[:P, j, c0:c0 + cn],
                            start=(s == 0), stop=(s == last))
                        nc.tensor.matmul(
                            num_p[:, c0:c0 + cn], lhsT,
                            ep_t[:P, j, c0:c0 + cn],
                            start=(s == 0), stop=(s == last))
            trip = finish_sb(den_p, num_p, sum(SBS[cur_sb]), trip)

    nc.compile()
    return nc


def _host_tables(x, tpx_bias, bias_X):
    import ml_dtypes

    t = (np.asarray(x, np.float32) + np.asarray(tpx_bias, np.float32)[0])
    r = t[0]
    for n in range(1, F_P):
        r = r[..., None] * t[n]                      # [12,12,12,12]
    px = _softmax_last(r + np.asarray(bias_X, np.float32))
    pxT = np.ascontiguousarray(px.transpose(3, 0, 1, 2)).reshape(K, FREE)
    pxr = np.ascontiguousarray(np.tile(pxT, (TPT, 1))).astype(ml_dtypes.bfloat16)

    W = np.zeros((13, ROWS_FULL, ROWS_FULL), np.float32)
    for s in range(12):
        for t_ in range(TPT):
            W[s, t_ * K:(t_ + 1) * K, 10 * s + t_] = 1.0
    for t_ in range(6):
        W[12, t_ * K:(t_ + 1) * K, 90 + t_] = 1.0
    wst = np.ascontiguousarray(W.transpose(1, 0, 2)).astype(ml_dtypes.bfloat16)
    return pxr, wst


def _shard_inputs(bias_Y_given_X):
    """Per-core [120, 22*1728] bf16: partition = t_local*12 + f4, free =
    concatenated tile f123 planes (tile 21 zero-padded past 6 triples)."""
    import ml_dtypes

    B7 = np.ascontiguousarray(np.asarray(bias_Y_given_X, np.float32)).reshape(
        NTRIP, K, K, K, K)
    tpts = [tpt for slots in SBS for tpt in slots]
    xs = []
    for c in range(N_CORES):
        shard = B7[c * TPC:(c + 1) * TPC]            # [216, f1,f2,f3,f4]
        rows = np.ascontiguousarray(shard.transpose(0, 4, 1, 2, 3)).reshape(
            TPC * K, FREE)                           # row = t*12+f4
        flat = np.zeros((ROWS_FULL, NTILES, FREE), np.float32)
        r = 0
        for ti, tpt in enumerate(tpts):
            P = tpt * K
            flat[:P, ti] = rows[r:r + P]
            r += P
        xs.append(flat.reshape(ROWS_FULL, NTILES * FREE)
                  .astype(ml_dtypes.bfloat16))
    return xs


def _make_inmaps(x, tpx_bias, bias_X, bias_Y_given_X):
    pxr, wst = _host_tables(x, tpx_bias, bias_X)
    xs = _shard_inputs(bias_Y_given_X)
    return [{"xin": xs[c], "pxr": pxr, "wst": wst} for c in range(N_CORES)]


def kernel(x, context_x, context_y, H_bandwidth, tpx_bias, bias_Y_given_X,
           bias_X):
    from concourse.bass_utils import run_bass_kernel_spmd

    if "nc" not in _CACHE:
        _CACHE["nc"] = _build_program()
    nc = _CACHE["nc"]

    in_maps = _make_inmaps(x, tpx_bias, bias_X, bias_Y_given_X)

    res = run_bass_kernel_spmd(nc, in_maps, list(range(N_CORES)))
    m_flat = np.concatenate(
        [np.asarray(res.results[c]["mout"], np.float32)[:, 0]
         for c in range(N_CORES)])
    m_k = m_flat.reshape(K, K, K)

    marginals = np.stack([
        m_k.sum(axis=(1, 2)), m_k.sum(axis=(0, 2)), m_k.sum(axis=(0, 1))
    ]).astype(np.float32)
    return _softmax_last(marginals).astype(np.float32)
